# revision 68
# baseline (speedup 1.0000x reference)
"""Trainium2 Bass kernel for AdaptiveAdjacencyMatrix.

Math: reference computes S = renorm(mask * softmax_j(proj_i + proj_j + b))
with proj = h @ w.  Inside a row softmax the proj_i and b terms cancel, so
every valid row i < size_b of S[b] is the SAME vector
    v_b[j] = exp(proj_j) * mask_j / sum_j'(exp(proj_j') * mask_j')
and all other entries (rows i >= size_b, columns j >= size_b) are exactly
zero.  The device computes, per batch: a matvec (PE), exp + row-sum (ACT),
1/Z (DVE), an outer-product broadcast of v to 128 partitions (PE), and
writes ceil(size/128) copies of the [128, size] tile; the host assembles
the full (B, M, M) output, filling the exact zeros of the padded region.

Ragged specialization: sizes are host-visible before compile, so the
program is compiled for this call's sizes.  Batches are snake-dealt to the
8 cores by sorted size so every core's slot-k batch has (nearly) the same
size; slot k is compiled with width W_k = max over cores (rounded up), and
padded columns j in [size, W_k) of h are filled host-side with
-30 * w / ||w||^2 so their logit is exactly -30 (exp ~ 1e-13, i.e. they
drop out of the sum and the stored values there are ignored by the host).

Sharding: data-parallel over batch B=32 across 8 cores (4 per core), no
collectives.  Loads go on the sync (SP) HWDGE ring, stores on the scalar
(ACT) ring, so stores never queue behind loads.
"""

import numpy as np

_CORES = 8
_B, _M, _H = 32, 1024, 512
_BLOC = _B // _CORES  # 4 batches (slots) per core
_NCHUNK = _H // 128  # 4 contraction chunks

_cache = {}


def _get_nc(Ws, Ts):
    """Compile (and cache) the SPMD program for slot widths Ws / tile
    counts Ts (slot order = ascending width)."""
    key = ("nc", tuple(Ws), tuple(Ts))
    if key in _cache:
        return _cache[key]

    import concourse.bacc as bacc
    import concourse.bass as bass
    import concourse.mybir as mybir
    import concourse.tile as tile

    f32 = mybir.dt.float32
    DT = mybir.dt.bfloat16
    Exp = mybir.ActivationFunctionType.Exp
    add = mybir.AluOpType.add

    nc = bacc.Bacc(
        "TRN2",
        target_bir_lowering=False,
        debug=False,
        enable_partition_id=False,
    )

    # per-slot inputs hT[p, j, c] = h[b, j, c*128 + p], j < W_k
    h_ext = [
        nc.declare_dram_parameter(f"h{k}", [128, Ws[k], _NCHUNK], DT, isOutput=False)
        for k in range(_BLOC)
    ]
    w4_ext = nc.declare_dram_parameter("w4", [128, _NCHUNK], DT, isOutput=False)
    # out{k}[p, t*W + j] = v[j] (t-th replica); one contiguous store per slot
    out_ext = [
        nc.declare_dram_parameter(f"out{k}", [128, Ts[k] * Ws[k]], DT, isOutput=True)
        for k in range(_BLOC)
    ]

    with tile.TileContext(nc) as tc:
        with (
            tc.tile_pool(name="const", bufs=1) as const_pool,
            tc.tile_pool(name="hbuf", bufs=1) as h_pool,
            tc.tile_pool(name="vbuf", bufs=1) as v_pool,
            tc.tile_pool(name="small", bufs=1) as small_pool,
            tc.tile_pool(name="psp", bufs=3, space="PSUM") as psum_proj,
            tc.tile_pool(name="pso", bufs=4, space="PSUM") as psum_out,
        ):
            w_sb = const_pool.tile([128, _NCHUNK], DT)
            nc.sync.dma_start(w_sb[:], w4_ext[:])
            zbias = const_pool.tile([1, 1], f32)
            nc.vector.memset(zbias[:], 0.0)
            ones_sb = const_pool.tile([1, 128], DT)
            nc.vector.memset(ones_sb[:], 1.0)

            # ---- all input loads up front on the sync ring, split at 512
            # so slot 0's first proj piece starts at ~half its load ----
            h_tiles = []
            for k in range(_BLOC):
                W = Ws[k]
                hT_t = h_pool.tile([128, W, _NCHUNK], DT, tag=f"h{k}")
                if W > 512:
                    nc.sync.dma_start(hT_t[:, 0:512, :], h_ext[k][:, 0:512, :])
                    nc.sync.dma_start(hT_t[:, 512:W, :], h_ext[k][:, 512:W, :])
                else:
                    nc.sync.dma_start(hT_t[:], h_ext[k][:])
                h_tiles.append(hT_t)

            for k in range(_BLOC):
                W, T = Ws[k], Ts[k]
                hT_t = h_tiles[k]
                pieces = [(0, min(512, W))]
                if W > 512:
                    pieces.append((512, W - 512))
                np_ = len(pieces)

                e_t = small_pool.tile([1, W], DT, tag=f"e{k}")
                zacc = small_pool.tile([1, np_], f32, tag=f"za{k}")
                for pi, (j0, pw) in enumerate(pieces):
                    pp = psum_proj.tile([1, pw], f32, tag="proj")
                    for c in range(_NCHUNK):
                        nc.tensor.matmul(
                            pp[:],
                            w_sb[:, c : c + 1],
                            hT_t[:, j0 : j0 + pw, c],
                            start=(c == 0),
                            stop=(c == _NCHUNK - 1),
                        )
                    # e = exp(proj); accum_out gives the partial row sum
                    nc.scalar.activation(
                        e_t[0:1, j0 : j0 + pw],
                        pp[:],
                        Exp,
                        bias=zbias[:],
                        accum_out=zacc[0:1, pi : pi + 1],
                    )
                # 1/Z, broadcast to a [1, 128] row for the outer product
                rz = small_pool.tile([1, 1], f32, tag=f"rz{k}")
                if np_ > 1:
                    zsum = small_pool.tile([1, 1], f32, tag=f"zs{k}")
                    nc.vector.tensor_reduce(
                        zsum[:], zacc[:], mybir.AxisListType.X, add
                    )
                    nc.vector.reciprocal(rz[:], zsum[:])
                else:
                    nc.vector.reciprocal(rz[:], zacc[:])
                rzrow = small_pool.tile([1, 128], DT, tag=f"rr{k}")
                nc.vector.tensor_scalar_mul(rzrow[:], ones_sb[:], rz[:])

                # vb[p, j] = rz * e_j for all p (outer product via PE).
                # high_priority: the scheduler must order these ahead of the
                # NEXT slot's proj matmuls on the PE queue, else each store
                # is delayed ~4us and the store ring starves mid-kernel
                vb_sb = v_pool.tile([128, W], DT, tag=f"vb{k}")
                with tc.high_priority():
                    for pi, (j0, pw) in enumerate(pieces):
                        ps = psum_out.tile([128, pw], f32, tag="vb")
                        nc.tensor.matmul(
                            ps[:],
                            rzrow[:],
                            e_t[0:1, j0 : j0 + pw],
                            start=True,
                            stop=True,
                        )
                        nc.vector.tensor_copy(vb_sb[:, j0 : j0 + pw], ps[:])

                    # one store per slot on the scalar (ACT) HWDGE ring;
                    # the source AP repeats the same [128, W] tile T times
                    # (stride-0 middle dim) so the DMA does the replication
                    vb_ap = vb_sb[:]
                    rep_ap = bass.AP(
                        vb_ap.tensor,
                        vb_ap.offset,
                        [vb_ap.ap[0], [0, T], vb_ap.ap[1]],
                    )
                    nc.scalar.dma_start(out_ext[k][:], rep_ap)

    nc.compile()
    _cache[key] = nc
    return nc


def _np_dt():
    import ml_dtypes

    return np.dtype(ml_dtypes.bfloat16)


def _ensure_ntff_hook():
    """Install the axon NTFF profiling hook if the image's antenv lacks it.

    Mirrors trn_boot._ntff_profile_via_ctypes: drives NRT profiling via the
    libaxon_pjrt.so C ABI so run_bass_kernel_spmd(trace=True) can report
    exec_time_ns.  No-op if anything is missing.
    """
    import contextlib
    import ctypes
    import os
    import sys
    import types

    try:
        from antenv.axon_hooks import get_axon_ntff_profile_hook

        if get_axon_ntff_profile_hook() is not None:
            return
        have_mod = True
    except ImportError:
        have_mod = False

    so_path = "/opt/axon/libaxon_pjrt.so"
    if not os.path.exists(so_path):
        return
    lib = ctypes.CDLL(so_path)
    if not hasattr(lib, "axon_start_nrt_profile"):
        return
    lib.axon_start_nrt_profile.argtypes = [
        ctypes.POINTER(ctypes.c_int64),
        ctypes.c_size_t,
    ]
    lib.axon_start_nrt_profile.restype = ctypes.c_int64
    lib.axon_stop_nrt_profile.argtypes = [ctypes.c_char_p]
    lib.axon_stop_nrt_profile.restype = ctypes.c_int64

    @contextlib.contextmanager
    def _hook(output_dir, device_ids):
        import jax

        jax.devices()
        if device_ids:
            ids = (ctypes.c_int64 * len(device_ids))(*device_ids)
            rc = lib.axon_start_nrt_profile(ids, len(device_ids))
        else:
            rc = lib.axon_start_nrt_profile(None, 0)
        if rc != 0:
            raise RuntimeError(f"axon_start_nrt_profile rc={rc}")
        try:
            yield
        finally:
            n = lib.axon_stop_nrt_profile(str(output_dir).encode())
            print(f"ntff profile: {n} file(s) written to {output_dir}")

    if have_mod:
        from antenv import axon_hooks

        axon_hooks.set_axon_ntff_profile_hook(_hook)
    else:
        mod = types.ModuleType("antenv.axon_hooks")
        state = {"hook": _hook}
        mod.get_axon_ntff_profile_hook = lambda: state["hook"]
        mod.set_axon_ntff_profile_hook = lambda h: state.__setitem__("hook", h)
        sys.modules["antenv.axon_hooks"] = mod


def _run_with_retry(nc, in_maps, trace, attempts=3):
    """Retry transient device errors (NRT_EXEC_UNIT_UNRECOVERABLE has been
    observed to clear on re-execution)."""
    import time

    from concourse.bass_utils import run_bass_kernel_spmd

    for a in range(attempts):
        try:
            return run_bass_kernel_spmd(
                nc, in_maps, core_ids=list(range(_CORES)), trace=trace
            )
        except Exception:
            if a == attempts - 1:
                raise
            time.sleep(8)


def kernel(h, w, b, original_sizes, _trace=False):
    if _trace:
        _ensure_ntff_hook()
    dt = _np_dt()

    h = np.asarray(h, dtype=np.float32)
    w = np.asarray(w, dtype=np.float32)
    sizes = np.asarray(original_sizes).astype(np.int64)
    sizes = np.clip(sizes, 1, _M)
    assert h.shape == (_B, _M, _H)

    # ---- snake-deal batches to (core, slot) by descending size so each
    # slot's size is nearly uniform across cores ----
    ranks = np.argsort(-sizes, kind="stable")
    assign = np.empty((_CORES, _BLOC), dtype=np.int64)  # [core, slot] -> batch
    for k in range(_BLOC):
        for c in range(_CORES):
            pos = c if k % 2 == 0 else _CORES - 1 - c
            assign[c, k] = ranks[k * _CORES + pos]
    # per-slot static width/tile-count (max over cores).  Order: smallest
    # slot first (earliest possible first store), then descending so the
    # big stores drain while the remaining slots compute.
    smax = sizes[assign].max(axis=0)  # [slot]
    desc = list(np.argsort(-smax, kind="stable"))
    order = np.array(desc[-1:] + desc[:-1])
    assign = assign[:, order]
    smax = smax[order]
    Ws = [int(min(_M, -(-int(s) // 16) * 16)) for s in smax]
    Ts = [int(-(-int(s) // 128)) for s in smax]

    nc = _get_nc(Ws, Ts)

    # ---- host-side input prep ----
    w4 = np.ascontiguousarray(w.reshape(_NCHUNK, 128).T).astype(dt)  # (128, 4)
    pad = (-30.0 / float(np.dot(w, w))) * w  # proj(pad) == -30 exactly
    # hT[b][p, j, c] = h[b, j, c*128+p]; (B, 128, M, NCHUNK)
    hT = np.ascontiguousarray(
        h.reshape(_B, _M, _NCHUNK, 128).transpose(0, 3, 1, 2)
    ).astype(dt)
    padT = pad.reshape(_NCHUNK, 128).T.astype(dt)  # (128, NCHUNK)

    in_maps = []
    for c in range(_CORES):
        m = {"w4": w4}
        for k in range(_BLOC):
            bidx = int(assign[c, k])
            s = int(sizes[bidx])
            W = Ws[k]
            hk = np.empty((128, W, _NCHUNK), dtype=dt)
            hk[:, :s, :] = hT[bidx, :, :s, :]
            if W > s:
                hk[:, s:, :] = padT[:, None, :]
            m[f"h{k}"] = hk
        in_maps.append(m)

    res = _run_with_retry(nc, in_maps, trace=_trace)
    _cache["last_result"] = res

    # ---- assemble full output; padded region is exactly zero ----
    out = np.zeros((_B, _M, _M), dtype=np.float32)
    for c in range(_CORES):
        for k in range(_BLOC):
            bidx = int(assign[c, k])
            s = int(sizes[bidx])
            blk = np.asarray(res.results[c][f"out{k}"])  # [128, T*W]
            blk = (
                blk.reshape(128, Ts[k], Ws[k])
                .transpose(1, 0, 2)
                .reshape(Ts[k] * 128, Ws[k])
            )
            out[bidx, :s, :s] = blk[:s, :s].astype(np.float32)
    return out


def last_exec_time_ns():
    res = _cache.get("last_result")
    return None if res is None else res.exec_time_ns
